# revision 1
# baseline (speedup 1.0000x reference)
"""Multi-head attention (B=2, S=2048, D=1024, H=16) on 8 Trainium2 cores.

Sharding: batch x head-group. Core c handles batch b = c//4 and heads
4*(c%4) .. 4*(c%4)+4 (a 256-wide slice of the feature dim). Each core:
  - projects q/k/v for its batch with its weight slices (transposed
    projections qhT/khT [d, s]; vh natural [s, d]),
  - computes scores transposed [k, q] on PE (f32r, full speed), two heads
    row-packed in the 128x128 array (head_dim=64),
  - softmax: exp on ScalarE (no max subtraction needed: |scores| <= ~0.7
    by construction), denominator via a ones-column appended to vh so the
    attnV matmul emits it as row 64 of the output,
  - attn @ V accumulated over k-tiles in PSUM, normalized at evacuation,
  - output projection into a partial y [2048, 1024], summed on host.

All matmuls use float32r (FP22 multiply, fp32 accumulate): 1 cycle/row on
TRN2 for moving free dim >= 256, vs 4 cycles/row for full fp32.
"""

import copy
import sys

import numpy as np

if "/opt/trn_rl_repo" not in sys.path:
    sys.path.insert(0, "/opt/trn_rl_repo")

B = 2
S = 2048
DIM = 1024
NH = 16
HD = 64
NCORES = 8
GROUPS = NCORES // B          # 4 head-groups per batch
HPC = NH // GROUPS            # 4 heads per core
CS = HPC * HD                 # 256-wide feature slice per core
PAIRS = HPC // 2              # head pairs per core

_PROGRAM = None


def _split_waits(nc, templates, max_waits=1):
    """This walrus build rejects instructions carrying more than one sync-wait
    command (verified for MATMUL/LDW, ACTIVATE, DMACopy and CTRL structs
    alike). Move excess waits onto injected same-engine NOPs placed right
    before the over-subscribed instruction (conditions are checked in the same
    engine-stream position, so semantics are unchanged)."""
    from concourse import mybir

    n_nops = 0
    for f in nc.m.functions:
        for blk in f.blocks:
            insts = blk.instructions
            i = 0
            while i < len(insts):
                inst = insts[i]
                si = inst.sync_info
                if si is not None and si.on_wait and len(si.on_wait) > max_waits:
                    waits = list(si.on_wait)
                    keep = waits[-max_waits:]
                    extra = waits[:-max_waits]
                    nops = []
                    for w in extra:
                        nop = copy.deepcopy(templates[inst.engine])
                        nop.name = f"waitnop-{n_nops}"
                        nop.sync_info = mybir.SyncInfo(on_wait=[w], on_update=[])
                        nops.append(nop)
                        n_nops += 1
                    inst.sync_info = mybir.SyncInfo(
                        on_wait=keep, on_update=list(si.on_update))
                    insts[i:i] = nops
                    i += len(nops)
                i += 1
    return n_nops


def _build_program(split=True, phases=('qk', 'v', 'att', 'fin'), loop_n=1):
    import concourse.bass as bass
    import concourse.tile as tile
    from concourse import mybir

    dt = mybir.dt
    f32 = dt.float32
    f32r = dt.float32r
    f16 = dt.float16
    ACT = mybir.ActivationFunctionType

    nc = bass.Bass()
    nop_templates = {
        eng.engine: eng.nop().ins
        for eng in (nc.tensor, nc.vector, nc.scalar, nc.gpsimd, nc.sync)
    }

    xqT = nc.dram_tensor("xqT", [DIM, S], f16, kind="ExternalInput")
    xkT = nc.dram_tensor("xkT", [DIM, S], f16, kind="ExternalInput")
    xvT = nc.dram_tensor("xvT", [DIM, S], f16, kind="ExternalInput")
    wqT = nc.dram_tensor("wqT", [DIM, CS], f16, kind="ExternalInput")
    wkT = nc.dram_tensor("wkT", [DIM, CS], f16, kind="ExternalInput")
    wvT = nc.dram_tensor("wvT", [DIM, CS], f16, kind="ExternalInput")
    woT = nc.dram_tensor("woT", [CS, DIM], f32r, kind="ExternalInput")
    bq_s = nc.dram_tensor("bq_s", [CS, 1], f32, kind="ExternalInput")
    bk_s = nc.dram_tensor("bk_s", [CS, 1], f32, kind="ExternalInput")
    bv_s = nc.dram_tensor("bv_s", [1, CS], f32, kind="ExternalInput")
    ones_c = nc.dram_tensor("ones_c", [1, (S // 128) * HPC], f32r, kind="ExternalInput")
    y = nc.dram_tensor("y", [S, DIM], f32, kind="ExternalOutput")

    KT_PROJ = DIM // 128      # 8 contraction tiles for projections
    KT_ATT = S // 128         # 16 k-position tiles for attention
    ST = S // 128             # 16 s-tiles
    SCALE = 1.0 / np.sqrt(np.float32(DIM))

    import contextlib

    with tile.TileContext(nc) as tc:
        loop_cm = (
            tc.For_i(0, loop_n, 1, hint_engines=(
                mybir.EngineType.PE, mybir.EngineType.Activation,
                mybir.EngineType.DVE, mybir.EngineType.SP))
            if loop_n > 1 else contextlib.nullcontext())
        with (
            loop_cm,
            tc.tile_pool(name="weights", bufs=1) as wpool,
            tc.tile_pool(name="persist", bufs=1) as persist,
            tc.tile_pool(name="xstream", bufs=4) as xpool,
            tc.tile_pool(name="xvstream", bufs=3) as xvpool,
            tc.tile_pool(name="exp", bufs=3) as expool,
            tc.tile_pool(name="small", bufs=2) as spool,
            tc.tile_pool(name="yout", bufs=3) as ypool,
        ):
            # ---- SBUF allocations ----
            wq_sb = wpool.tile([128, KT_PROJ, CS], f16, tag="wq")
            wk_sb = wpool.tile([128, KT_PROJ, CS], f16, tag="wk")
            wv_sb = wpool.tile([128, KT_PROJ, CS], f16, tag="wv")
            wo_sb = wpool.tile([128, PAIRS, DIM], f32r, tag="wo")
            bq_sb = wpool.tile([128, PAIRS], f32, tag="bq")
            bk_sb = wpool.tile([128, PAIRS], f32, tag="bk")
            vb_sb = wpool.tile([128, CS], f32, tag="vb")
            qhT = persist.tile([128, PAIRS, S], f32r, tag="qhT")
            khT = persist.tile([128, PAIRS, S], f32r, tag="khT")
            # vh: per s-tile, per head: 64 cols of v plus a ones column (the
            # softmax denominator emerges as row 64 of the attnV output).
            vh = persist.tile([128, ST, HPC, HD + 1], f32r, tag="vh")
            out_sT = persist.tile([128, PAIRS, S], f32r, tag="out_sT")

            # ---- early loads: only what Q/K projection needs immediately.
            # Weight k-tiles are interleaved with the x-stream inside the
            # projection loop; wv/wo/vb/ones are DMA'd later so the x-streams
            # (the startup critical path) aren't queued behind them.
            for pr in range(PAIRS):
                nc.sync.dma_start(bq_sb[:, pr:pr + 1], bq_s[pr * 128:(pr + 1) * 128, :])
                nc.sync.dma_start(bk_sb[:, pr:pr + 1], bk_s[pr * 128:(pr + 1) * 128, :])

            if 'qk' in phases:
                # ---- Q/K projections (transposed outputs, head-pair layout) ----
                for (xT, wT, w_sb, out_sb, bias_sb) in (
                    (xqT, wqT, wq_sb, qhT, bq_sb),
                    (xkT, wkT, wk_sb, khT, bk_sb),
                ):
                    with tc.tile_pool(name="qkpsum", bufs=2, space="PSUM") as qkp:
                        ps = [qkp.tile([128, S], f32, tag="qk", name=f"qkps{i}") for i in range(PAIRS)]
                        for kt in range(KT_PROJ):
                            nc.sync.dma_start(w_sb[:, kt, :],
                                              wT[kt * 128:(kt + 1) * 128, :])
                            xt = xpool.tile([128, S], f16, tag="xt")
                            nc.sync.dma_start(xt[:], xT[kt * 128:(kt + 1) * 128, :])
                            for pr in range(PAIRS):
                                for qc in range(S // 512):
                                    nc.tensor.matmul(
                                        ps[pr][:, qc * 512:(qc + 1) * 512],
                                        w_sb[:, kt, pr * 128:(pr + 1) * 128],
                                        xt[:, qc * 512:(qc + 1) * 512],
                                        start=(kt == 0), stop=(kt == KT_PROJ - 1),
                                    )
                        for pr in range(PAIRS):
                            nc.scalar.activation(
                                out_sb[:, pr, :], ps[pr][:],
                                ACT.Identity, bias=bias_sb[:, pr:pr + 1], scale=1.0)

            # ---- late loads: land while the QK x-streams / attention run ----
            for kt in range(KT_PROJ):
                nc.sync.dma_start(wv_sb[:, kt, :], wvT[kt * 128:(kt + 1) * 128, :])
            nc.sync.dma_start(vb_sb[:], bv_s[:].to_broadcast((128, CS)))
            nc.sync.dma_start(
                vh[:, :, :, HD:HD + 1],
                ones_c[:].to_broadcast((128, ST * HPC)))
            for pr in range(PAIRS):
                nc.sync.dma_start(wo_sb[:, pr, :], woT[pr * 128:(pr + 1) * 128, :])

            # ---- V projection + attention share one PSUM pool so the V
            # matmuls/evacs can overlap attention's PE/DVE slack:
            #   vp 1 bank x2 + sc 2 banks x2 + at 1 bank x2 = 8 banks.
            vap = tc.alloc_tile_pool(name="vattps", bufs=2, space="PSUM")
            dnrpool = tc.alloc_tile_pool(name="dnr", bufs=2, space="DRAM")

            if 'v' in phases:
                # ---- V projection (natural [s, d] layout + bias + ones) ----
                xvT_r = xvT.rearrange("(t p) (s c) -> p t s c", p=128, c=256)
                for st2 in range(ST // 2):
                    xvt = xvpool.tile([128, KT_PROJ, 256], f16, tag="xv")
                    nc.sync.dma_start(xvt[:], xvT_r[:, :, st2, :])
                    for sub in range(2):
                        st = st2 * 2 + sub
                        vp = vap.tile([128, CS], f32, tag="vp")
                        for kt in range(KT_PROJ):
                            nc.tensor.matmul(
                                vp[:], xvt[:, kt, sub * 128:(sub + 1) * 128],
                                wv_sb[:, kt, :],
                                start=(kt == 0), stop=(kt == KT_PROJ - 1))
                        nc.vector.tensor_add(
                            vh[:, st, :, 0:HD],
                            vp[:].rearrange("p (h c) -> p h c", c=HD),
                            vb_sb[:].rearrange("p (h c) -> p h c", c=HD))

            if 'att' in phases:
                # ---- attention: per (512-wide q chunk, head pair) ----
                # Scores PSUM is double-buffered (pool bufs=2), so scores for
                # k-tile kt+1 run while ScalarE exps k-tile kt; the loop is
                # paced by the exp at ~1.04us per [128, 1024] op. After both
                # pairs finish a q chunk, the output projection for those four
                # s-tiles is emitted so it (and the y DMA-out) overlaps the
                # next chunk's attention.
                for qck in range(S // 512):
                    for pr in range(PAIRS):
                        q0 = qck * 512
                        at = [vap.tile([HD + 1, 512], f32, tag="at",
                                       name=f"at{i}") for i in range(2)]

                        def scores(kt):
                            # both heads of the pair, row-packed in PE
                            sct = vap.tile([128, 1024], f32, tag="sc",
                                           name="sct")
                            for h in range(2):
                                nc.tensor.matmul(
                                    sct[:, h * 512:(h + 1) * 512],
                                    khT[h * 64:(h + 1) * 64, pr,
                                        kt * 128:(kt + 1) * 128],
                                    qhT[h * 64:(h + 1) * 64, pr,
                                        q0:q0 + 512],
                                    start=True, stop=True)
                            return sct

                        sc_cur = scores(0)
                        for kt in range(KT_ATT):
                            ex = expool.tile([128, 1024], f32r, tag="ex")
                            nc.scalar.activation(ex[:], sc_cur[:], ACT.Exp,
                                                 scale=SCALE)
                            if kt + 1 < KT_ATT:
                                sc_cur = scores(kt + 1)
                            for h in range(2):
                                nc.tensor.matmul(
                                    at[h][:],
                                    vh[:, kt, pr * 2 + h, :],
                                    ex[:, h * 512:(h + 1) * 512],
                                    start=(kt == 0), stop=(kt == KT_ATT - 1),
                                    skip_group_check=True)

                        # Evacuate both heads: DVE-copy the [65, 512] PSUM
                        # tiles to SBUF staging (releases the banks fast),
                        # reciprocal the denominator rows, bounce them through
                        # DRAM to broadcast across partitions, then normalize.
                        # Head 1's result is partition-shifted into rows
                        # 64..127 of out_sT by an SBUF->SBUF DMA.
                        stg = [spool.tile([HD + 1, 512], f32, tag="stg",
                                          name=f"stg{i}", bufs=4)
                               for i in range(2)]
                        rc = spool.tile([HD + 1, 1024], f32, tag="recip")
                        for h in range(2):
                            nc.vector.tensor_copy(stg[h][:], at[h][:])
                            nc.vector.reciprocal(
                                rc[HD:HD + 1, h * 512:(h + 1) * 512],
                                stg[h][HD:HD + 1, :])
                        dnr = dnrpool.tile([1, 1024], f32, tag="dnr")
                        nc.sync.dma_start(dnr[:], rc[HD:HD + 1, :])
                        rb = [spool.tile([64, 512], f32, tag="rb",
                                         name=f"rb{i}", bufs=4)
                              for i in range(2)]
                        for h in range(2):
                            nc.sync.dma_start(
                                rb[h][:],
                                dnr[0:1, h * 512:(h + 1) * 512]
                                .to_broadcast((64, 512)))
                        nc.vector.tensor_mul(
                            out_sT[0:64, pr, q0:q0 + 512],
                            stg[0][0:HD, :], rb[0][:])
                        tmp = spool.tile([64, 512], f32r, tag="tmp")
                        nc.vector.tensor_mul(tmp[:], stg[1][0:HD, :], rb[1][:])
                        nc.sync.dma_start(
                            out_sT[64:128, pr, q0:q0 + 512], tmp[:])

                    if 'fin' in phases:
                        # ---- output projection for the four s-tiles whose
                        # out_sT columns this q chunk just completed ----
                        for st in range(4 * qck, 4 * qck + 4):
                            for n2 in range(DIM // 512):
                                yp = vap.tile([128, 512], f32, tag="vp",
                                              name="yp")
                                for pr in range(PAIRS):
                                    nc.tensor.matmul(
                                        yp[:],
                                        out_sT[:, pr, st * 128:(st + 1) * 128],
                                        wo_sb[:, pr, n2 * 512:(n2 + 1) * 512],
                                        start=(pr == 0), stop=(pr == PAIRS - 1))
                                ysb = ypool.tile([128, 512], f32, tag="ysb")
                                nc.vector.tensor_copy(ysb[:], yp[:])
                                nc.sync.dma_start(
                                    y[st * 128:(st + 1) * 128,
                                      n2 * 512:(n2 + 1) * 512], ysb[:])

            vap.release()
            dnrpool.release()

    nc.finalize()
    if split:
        _split_waits(nc, nop_templates)
    return nc


def _get_program():
    global _PROGRAM
    if _PROGRAM is None:
        _PROGRAM = _build_program()
    return _PROGRAM


def _make_in_maps(q, k, v, Wq, bq, Wk, bk, Wv, bv, Wo, bo):
    q = np.asarray(q, dtype=np.float32)
    k = np.asarray(k, dtype=np.float32)
    v = np.asarray(v, dtype=np.float32)
    Wq = np.asarray(Wq, dtype=np.float32)
    Wk = np.asarray(Wk, dtype=np.float32)
    Wv = np.asarray(Wv, dtype=np.float32)
    Wo = np.asarray(Wo, dtype=np.float32)
    bq = np.asarray(bq, dtype=np.float32)
    bk = np.asarray(bk, dtype=np.float32)
    bv = np.asarray(bv, dtype=np.float32)

    xT = {b: {
        "q": np.ascontiguousarray(q[b].T.astype(np.float16)),
        "k": np.ascontiguousarray(k[b].T.astype(np.float16)),
        "v": np.ascontiguousarray(v[b].T.astype(np.float16)),
    } for b in range(B)}

    in_maps = []
    for c in range(NCORES):
        b = c // GROUPS
        g = c % GROUPS
        hs = g * CS
        in_maps.append({
            "xqT": xT[b]["q"],
            "xkT": xT[b]["k"],
            "xvT": xT[b]["v"],
            "wqT": np.ascontiguousarray(Wq[hs:hs + CS, :].T.astype(np.float16)),
            "wkT": np.ascontiguousarray(Wk[hs:hs + CS, :].T.astype(np.float16)),
            "wvT": np.ascontiguousarray(Wv[hs:hs + CS, :].T.astype(np.float16)),
            "woT": np.ascontiguousarray(Wo[:, hs:hs + CS].T),
            "bq_s": np.ascontiguousarray(bq[hs:hs + CS].reshape(CS, 1)),
            "bk_s": np.ascontiguousarray(bk[hs:hs + CS].reshape(CS, 1)),
            "bv_s": np.ascontiguousarray(bv[hs:hs + CS].reshape(1, CS)),
            "ones_c": np.ones((1, 16 * 4), np.float32),
        })
    return in_maps


def _combine(results, bo):
    bo = np.asarray(bo, dtype=np.float32)
    out = np.zeros((B, S, DIM), np.float32)
    for c in range(NCORES):
        out[c // GROUPS] += results[c]["y"]
    out += bo
    return out


def run_on_hw(inputs, trace=False, **kwargs):
    """Run the kernel on the 8 NeuronCores. Returns (output, BassKernelResults)."""
    from concourse.bass_utils import run_bass_kernel_spmd

    nc = _get_program()
    in_maps = _make_in_maps(**inputs)
    res = run_bass_kernel_spmd(nc, in_maps, list(range(NCORES)),
                               trace=trace, **kwargs)
    return _combine(res.results, inputs["bo"]), res


def kernel(**inputs) -> np.ndarray:
    out, _ = run_on_hw(inputs, trace=False)
    return out

